# revision 2
# baseline (speedup 1.0000x reference)
"""Trainium2 Bass kernel for a 12-head attention block (B=2, N=2048, C=768).

Sharding: the 24 (batch, head) pairs are split across 8 NeuronCores —
4 cores per batch element, 3 heads per core (data + head/tensor parallel).
Each core computes qkv projections for its heads, the full attention for
its heads (the N x N score matrix is private to a core), and a *partial*
output projection over its heads' channels.  The host sums the 4 partial
projections per batch element (the tensor-parallel all-reduce) and adds
the bias.

Device algorithm (activations/weights bf16 — the PE streams its moving
operand at 1 column/cycle for 2-byte dtypes — with fp32 PSUM accumulation):

  xT [768, 2048] (x transposed on host)
  B:  qk^T  = W_qk^T.T @ xT  -> per-head [q^T(64); k^T(64)] x 2048 in PSUM
      (attention scale 1/8 and b_q, b_k folded into W/bias on host).
      The bias-add writes q^T into BOTH partition halves of qq[h] and k^T
      into BOTH halves of kk[h] (one half direct, the other via SBUF DMA):
      the paired S^T matmuls then read the SAME tile at the SAME column
      offsets from opposite halves, which lets the two contraction-64
      matmuls run CONCURRENTLY in opposite PE row groups (tile_position
      (0,0) / (64,0)) sharing one moving-operand stream.
  B2: v     = xT.T @ W_v^T   -> [2048, 3*65] with a column of ones per head
  C:  S^T[key, q] = kk.T @ qq          (per 128-key tile, 512-q chunk,
                                        emitted as overlapping row-group pairs)
      P^T = exp(S^T)                   (ScalarE, no max subtraction:
                                        logits are in [-3, 3] by construction)
      ctx_u^T[d|den, q] += [v | 1].T @ P^T   (fused denominator row)
  D:  ctx^T = ctx_u^T[0:64] * (1/den)  (den broadcast across partitions via a
      ones-row matmul at base partition 64, then reciprocal_approx_fast).
      Heads 0/1 land in one stacked [128, N] tile (h1 staged + DMA'd into
      partitions 64:128) so the projection contracts 128 deep.
  E:  y[n, :] += [ctx_h0; ctx_h1]^T @ wp01 + ctx_h2^T @ wp2
      (partial projection, summed on host)

Scheduling notes: the PE's HAM clock gate re-throttles 2.4->1.2 GHz after
~3.4us of idle, so the normalize/projection work for a tile is deferred
and emitted mid-way through the NEXT attention block; a warm-up matmul
spin bridges the input-DMA window; the exp table set is preloaded with a
dummy activation; x is DMA'd in q-chunk-column order so the first
projection group only waits for 1/4 of x.
"""

import numpy as np
import ml_dtypes

import concourse.bacc as bacc
import concourse.tile as tile
import concourse.mybir as mybir
from concourse.bass_utils import run_bass_kernel_spmd

# Problem shape (hardcoded; harness contract)
B, N, C = 2, 2048, 768
H, HD = 12, 64
NCORES = 8
CORES_PER_B = NCORES // B      # 4
HPC = H // CORES_PER_B         # 3 heads per core
P = 128
NT = N // P                    # 16 key/n tiles
KT = C // P                    # 6 c_in tiles
CH = 512                       # q chunk (max moving free dim)
QCH = N // CH                  # 4 chunks
VW = 3 * 65                    # v width: 3 heads x (64 + fused ones column)

f32 = mybir.dt.float32
bf16 = mybir.dt.bfloat16
EXP = mybir.ActivationFunctionType.Exp


def _emit(tc, nc, xT, w_qk, w_v, b_qk, w_p, vones, onesrow, y):
    from contextlib import ExitStack

    with ExitStack() as ctx:
        consts = ctx.enter_context(tc.tile_pool(name="consts", bufs=1))
        qq_pool = ctx.enter_context(tc.tile_pool(name="qq", bufs=HPC))
        kk_pool = ctx.enter_context(tc.tile_pool(name="kk", bufs=HPC))
        v_pool = ctx.enter_context(tc.tile_pool(name="v", bufs=NT))
        ctx_pool = ctx.enter_context(tc.tile_pool(name="ctxp", bufs=2))
        stage_pool = ctx.enter_context(tc.tile_pool(name="stage", bufs=2))
        y_pool = ctx.enter_context(tc.tile_pool(name="y", bufs=3))
        ps_a = ctx.enter_context(tc.tile_pool(name="ps_a", bufs=2, space="PSUM"))
        ps_s = ctx.enter_context(tc.tile_pool(name="ps_s", bufs=2, space="PSUM"))
        ps_c = ctx.enter_context(tc.tile_pool(name="ps_c", bufs=2, space="PSUM"))

        # ---- constants
        vones_sb = consts.tile([P, VW], bf16)
        nc.sync.dma_start(vones_sb[:], vones[:])
        wqk_sb = consts.tile([P, KT, 2 * HD * HPC], bf16)
        _wqk = w_qk.rearrange("(t p) m -> p t m", p=P)
        for kt in range(KT):
            nc.sync.dma_start(wqk_sb[:, kt, :], _wqk[:, kt, :])
        wv_sb = consts.tile([P, KT, VW], bf16)
        nc.sync.dma_start(wv_sb[:], w_v.rearrange("(t p) m -> p t m", p=P))
        bqk_sb = consts.tile([P, HPC], f32)
        nc.sync.dma_start(bqk_sb[:], b_qk.rearrange("t p -> p t"))
        # projection weights: heads 0+1 stacked for contraction-128 matmuls
        wp01_sb = consts.tile([P, C], bf16)
        nc.sync.dma_start(wp01_sb[:], w_p[0 : 2 * HD, :])
        wp2_sb = consts.tile([HD, C], bf16)
        nc.sync.dma_start(wp2_sb[:], w_p[2 * HD : 3 * HD, :])
        onesrow_sb = consts.tile([HD + 1, HD], bf16)
        nc.sync.dma_start(onesrow_sb[:], onesrow[:])

        # persistent activations: qq[h] holds q^T in BOTH partition halves,
        # kk[h] holds k^T in both halves (for row-group-paired S^T matmuls)
        qq_sb = [qq_pool.tile([P, N], bf16, tag="qq", name=f"qq{_}") for _ in range(HPC)]
        kk_sb = [kk_pool.tile([P, N], bf16, tag="kk", name=f"kk{_}") for _ in range(HPC)]
        v_sb = [v_pool.tile([P, VW], bf16, tag="v", name=f"v{_}") for _ in range(NT)]
        ctx01_sb = ctx_pool.tile([P, N], bf16, tag="ctx01", name="ctx01")
        ctx2_sb = ctx_pool.tile([HD, N], bf16, tag="ctx2", name="ctx2")

        x_pool = ctx.enter_context(tc.tile_pool(name="x", bufs=KT))
        x_sb = [x_pool.tile([P, N], bf16, tag="x", name=f"x{_}") for _ in range(KT)]
        # q-chunk-column DMA order: the first projection group (chunk 0)
        # only needs x[:, 0:512] of every c_in tile (~0.8MB, ~2.3us)
        for cc in range(QCH):
            for kt in range(KT):
                nc.sync.dma_start(
                    x_sb[kt][:, cc * CH : (cc + 1) * CH],
                    xT[kt * P : (kt + 1) * P, cc * CH : (cc + 1) * CH],
                )

        # PE warm-up: the HAM clock gate needs ~3.4us of sustained matmul
        # activity to lift the PE from 1.2 to 2.4 GHz, and re-throttles after
        # a ~3.4us idle window.  Spin dummy matmuls on the (tiny, early) vones
        # tile while the x/weight DMAs land so the real matmuls start warm.
        wps = ps_a.tile([P, CH], f32, tag="ps_a", name="warm_ps")
        for _ in range(55):
            nc.tensor.matmul(
                wps[:, 0:VW], vones_sb[:, 0:P], vones_sb[:], start=True, stop=True
            )
        # preload the exp spline table set (~2.7us) during the DMA window
        actwarm = consts.tile([P, VW], bf16)
        nc.scalar.activation(actwarm[:], vones_sb[:], EXP)

        def emit_qk_group(t, cc):
            # qk^T head tile t, q-chunk cc: [q^T(64); k^T(64)] x CH in psum,
            # then bias-add into the duplicated qq/kk layout
            sl = slice(cc * CH, (cc + 1) * CH)
            ps = ps_a.tile([P, CH], f32, tag="ps_a", name="ps_qk")
            for kt in range(KT):
                nc.tensor.matmul(
                    ps[:],
                    wqk_sb[:, kt, t * P : (t + 1) * P],
                    x_sb[kt][:, sl],
                    start=(kt == 0),
                    stop=(kt == KT - 1),
                )
            nc.vector.tensor_scalar_add(qq_sb[t][0:HD, sl], ps[0:HD, :], bqk_sb[0:HD, t : t + 1])
            nc.vector.tensor_scalar_add(kk_sb[t][HD:P, sl], ps[HD:P, :], bqk_sb[HD:P, t : t + 1])
            # duplicate into the opposite partition halves; chunked so the
            # SBUF->SBUF DMA overlaps the next chunk's matmuls
            nc.sync.dma_start(qq_sb[t][HD:P, sl], qq_sb[t][0:HD, sl])
            nc.sync.dma_start(kk_sb[t][0:HD, sl], kk_sb[t][HD:P, sl])

        def emit_v(nt):
            # v natural layout [key, 3*65] (+ ones columns)
            ps = ps_a.tile([P, CH], f32, tag="ps_a", name="ps_v")
            for kt in range(KT):
                nc.tensor.matmul(
                    ps[:, 0:VW],
                    x_sb[kt][:, nt * P : (nt + 1) * P],
                    wv_sb[:, kt, :],
                    start=(kt == 0),
                    stop=(kt == KT - 1),
                )
            nc.vector.tensor_add(v_sb[nt][:], ps[:, 0:VW], vones_sb[:])

        recip_pool = ctx.enter_context(tc.tile_pool(name="recip", bufs=2))
        bc_pool = ctx.enter_context(tc.tile_pool(name="bc", bufs=2))
        p_pool = ctx.enter_context(tc.tile_pool(name="p", bufs=8))

        # ---- C/D/E: attention, normalize (deferred one step), projection.
        # S^T pairs go to opposite PE row groups (partitions 0:64 / 64:128)
        # reading opposite halves of the SAME qq/kk tiles so their moving
        # streams coexist.  The normalize (D) for tile (c,h) is emitted only
        # after the next C-block so its reciprocal never stalls the PE.
        def emit_C(c, h, first=False, flush=None):
            cps = ps_c.tile([65, CH], f32, tag="ps_c", name="cps")
            for kp in range(NT // 2):
                kt0, kt1 = 2 * kp, 2 * kp + 1
                if first and kp >= 1:
                    # just-in-time v projection (v0/v1 are emitted before the
                    # C loop while the qq/kk dup DMA is still in flight)
                    emit_v(kt0)
                    emit_v(kt1)
                sps = ps_s.tile([P, 2 * CH], f32, tag="ps_s", name="sps")
                nc.tensor.matmul(
                    sps[:, 0:CH],
                    kk_sb[h][0:HD, kt0 * P : (kt0 + 1) * P],
                    qq_sb[h][0:HD, c * CH : (c + 1) * CH],
                )
                nc.tensor.matmul(
                    sps[:, CH : 2 * CH],
                    kk_sb[h][HD:P, kt1 * P : (kt1 + 1) * P],
                    qq_sb[h][HD:P, c * CH : (c + 1) * CH],
                )
                pt = p_pool.tile([P, 2 * CH], bf16, tag="p", name="pt")
                nc.scalar.activation(pt[:], sps[:], EXP)
                nc.tensor.matmul(
                    cps[:],
                    v_sb[kt0][:, h * 65 : (h + 1) * 65],
                    pt[:, 0:CH],
                    start=(kp == 0),
                    stop=False,
                )
                nc.tensor.matmul(
                    cps[:],
                    v_sb[kt1][:, h * 65 : (h + 1) * 65],
                    pt[:, CH : 2 * CH],
                    start=False,
                    stop=(kp == NT // 2 - 1),
                )
                if kp % 2 == 1 and flush is not None:
                    # emit the previous tile's normalize / projection units
                    # mid-block (not at kp 0: ACT has no buffered work at a
                    # block start, so PE detours there starve the exp stream)
                    flush(1)
            return cps

        def emit_D(c, h, cps):
            sl = slice(c * CH, (c + 1) * CH)
            denr = recip_pool.tile([65, CH], bf16, tag="denr", name="denr")
            nc.vector.tensor_copy(denr[64:65, :], cps[64:65, :])
            bps = ps_a.tile([P, CH], f32, tag="ps_a", name="bps")
            nc.tensor.matmul(
                bps[0:HD, :], onesrow_sb[HD : HD + 1, :], denr[64:65, :],
                start=True, stop=True,
            )
            bcd = bc_pool.tile([HD, CH], f32, tag="bcd", name="bcd")
            nc.vector.tensor_copy(bcd[:], bps[0:HD, :])
            bc = bc_pool.tile([HD, CH], f32, tag="bc", name="bc")
            nc.vector.reciprocal_approx_fast(bc[:], bcd[:])
            if h == 0:
                nc.vector.tensor_mul(ctx01_sb[0:HD, sl], cps[0:HD, :], bc[:])
            elif h == 1:
                # stage + DMA into partitions 64:128 of the stacked tile
                # (DVE lanes are partition-locked; only DMA can cross halves)
                stg = stage_pool.tile([HD, CH], bf16, tag="stg", name="stg")
                nc.vector.tensor_mul(stg[:], cps[0:HD, :], bc[:])
                nc.sync.dma_start(ctx01_sb[HD:P, sl], stg[:])
            else:
                nc.vector.tensor_mul(ctx2_sb[:, sl], cps[0:HD, :], bc[:])

        def emit_E_unit(nt):
            # partial projection for one n-tile (summed over this core's heads):
            # heads 0+1 contract 128 deep via the stacked ctx01 tile
            psA = ps_a.tile([P, CH], f32, tag="ps_a", name="psA")
            psB = ps_a.tile([P, CH], f32, tag="ps_a", name="psB")
            nsl = slice(nt * P, (nt + 1) * P)
            nc.tensor.matmul(psA[:], ctx01_sb[:, nsl], wp01_sb[:, 0:CH], start=True, stop=False)
            nc.tensor.matmul(psA[:], ctx2_sb[:, nsl], wp2_sb[:, 0:CH], start=False, stop=True)
            nc.tensor.matmul(psB[:, 0 : C - CH], ctx01_sb[:, nsl], wp01_sb[:, CH:C], start=True, stop=False)
            nc.tensor.matmul(psB[:, 0 : C - CH], ctx2_sb[:, nsl], wp2_sb[:, CH:C], start=False, stop=True)
            ysb = y_pool.tile([P, C], bf16, tag="y", name="ysb")
            nc.vector.tensor_copy(ysb[:, 0:CH], psA[:])
            nc.vector.tensor_copy(ysb[:, CH:C], psB[:, 0 : C - CH])
            nc.sync.dma_start(y[nt * P : (nt + 1) * P, :], ysb[:])

        pending = [None]  # (c, h, cps) awaiting D
        pending_E = []  # nt projection units ready to emit

        def flush_pending(budget=1):
            if pending[0] is not None:
                pc, ph, pcps = pending[0]
                emit_D(pc, ph, pcps)
                if ph == HPC - 1:
                    pending_E.extend(pc * (CH // P) + i for i in range(CH // P))
                pending[0] = None
                budget -= 1
            while budget > 0 and pending_E:
                emit_E_unit(pending_E.pop(0))
                budget -= 1

        # Head-outer schedule: all 4 q-chunks of head h, then head h+1.
        # The next head's qk projection is emitted one chunk-group at a time
        # underneath the current (ACT-bound) attention blocks.
        emit_qk_group(0, 0)
        emit_qk_group(0, 1)
        emit_v(0)
        emit_v(1)
        emit_qk_group(0, 2)
        emit_qk_group(0, 3)
        for h in range(HPC):
            for c in range(QCH):
                cps = emit_C(c, h, first=(h == 0 and c == 0), flush=flush_pending)
                pending[0] = (c, h, cps)
                if h < HPC - 1:
                    emit_qk_group(h + 1, c)
        while pending[0] is not None or pending_E:
            flush_pending(2)


def build_program():
    nc = bacc.Bacc("TRN2", target_bir_lowering=False, debug=False)
    xT = nc.dram_tensor("xT", [C, N], bf16, kind="ExternalInput").ap()
    w_qk = nc.dram_tensor("w_qk", [C, 2 * HD * HPC], bf16, kind="ExternalInput").ap()
    w_v = nc.dram_tensor("w_v", [C, VW], bf16, kind="ExternalInput").ap()
    b_qk = nc.dram_tensor("b_qk", [HPC, P], f32, kind="ExternalInput").ap()
    w_p = nc.dram_tensor("w_p", [HPC * HD, C], bf16, kind="ExternalInput").ap()
    vones = nc.dram_tensor("vones", [P, VW], bf16, kind="ExternalInput").ap()
    onesrow = nc.dram_tensor("onesrow", [HD + 1, HD], bf16, kind="ExternalInput").ap()
    y = nc.dram_tensor("y", [N, C], bf16, kind="ExternalOutput").ap()
    with tile.TileContext(nc) as tc:
        _emit(tc, nc, xT, w_qk, w_v, b_qk, w_p, vones, onesrow, y)
    nc.compile()
    return nc


_CACHE = {}


def _get_program():
    if "nc" not in _CACHE:
        _CACHE["nc"] = build_program()
    return _CACHE["nc"]


def make_in_maps(x, W_qkv, b_qkv, W_proj):
    """Per-core input dicts implementing the (batch, head-group) sharding."""
    x = np.ascontiguousarray(np.asarray(x, np.float32))
    W_qkv = np.asarray(W_qkv, np.float32)
    b_qkv = np.asarray(b_qkv, np.float32)
    W_proj = np.asarray(W_proj, np.float32)
    scale = float(HD) ** -0.5

    Wq = W_qkv[0:C].reshape(H, HD, C)
    Wk = W_qkv[C : 2 * C].reshape(H, HD, C)
    Wv = W_qkv[2 * C : 3 * C].reshape(H, HD, C)
    bq = b_qkv[0:C].reshape(H, HD)
    bk = b_qkv[C : 2 * C].reshape(H, HD)

    vones_mask = np.zeros((P, VW), np.float32)
    for i in range(HPC):
        vones_mask[:, i * 65 + HD] = 1.0
    onesrow_arr = np.zeros((HD + 1, HD), np.float32)
    onesrow_arr[HD, :] = 1.0

    in_maps = []
    for core in range(NCORES):
        b = core // CORES_PER_B
        hg = core % CORES_PER_B
        heads = list(range(hg * HPC, (hg + 1) * HPC))

        xT = np.ascontiguousarray(x[b].T).astype(ml_dtypes.bfloat16)  # [C, N]
        w_qk = np.empty((C, 2 * HD * HPC), np.float32)  # cast to bf16 below
        b_qk_arr = np.empty((HPC, P), np.float32)
        w_v = np.zeros((C, VW), np.float32)
        w_p = np.empty((HPC * HD, C), np.float32)
        for i, h in enumerate(heads):
            w_qk[:, i * P : i * P + HD] = Wq[h].T * scale
            w_qk[:, i * P + HD : (i + 1) * P] = Wk[h].T
            b_qk_arr[i, 0:HD] = bq[h] * scale
            b_qk_arr[i, HD:P] = bk[h]
            w_v[:, i * 65 : i * 65 + HD] = Wv[h].T
            w_p[i * HD : (i + 1) * HD, :] = W_proj[:, h * HD : (h + 1) * HD].T
        in_maps.append(
            {"xT": xT,
             "w_qk": w_qk.astype(ml_dtypes.bfloat16),
             "w_v": w_v.astype(ml_dtypes.bfloat16),
             "b_qk": b_qk_arr,
             "w_p": w_p.astype(ml_dtypes.bfloat16),
             "vones": vones_mask.astype(ml_dtypes.bfloat16),
             "onesrow": onesrow_arr.astype(ml_dtypes.bfloat16)}
        )
    return in_maps


def gather_output(results, b_qkv, W_proj, b_proj):
    """Sum the per-core partial projections (TP all-reduce) + effective bias."""
    out = np.zeros((B, N, C), np.float32)
    for core in range(NCORES):
        out[core // CORES_PER_B] += np.asarray(results[core]["y"], np.float32)
    b_v = np.asarray(b_qkv, np.float32)[2 * C : 3 * C]
    b_eff = np.asarray(b_proj, np.float32) + np.asarray(W_proj, np.float32) @ b_v
    out += b_eff
    return out


def kernel(x=None, xpos=None, W_qkv=None, b_qkv=None, W_proj=None, b_proj=None, **kw):
    del xpos, kw  # rope disabled in this configuration; xpos unused
    nc = _get_program()
    in_maps = make_in_maps(x, W_qkv, b_qkv, W_proj)
    res = run_bass_kernel_spmd(nc, in_maps, core_ids=list(range(NCORES)))
    return gather_output(res.results, b_qkv, W_proj, b_proj)


# revision 3
# speedup vs baseline: 1.1831x; 1.1831x over previous
"""Trainium2 Bass kernel for a 12-head attention block (B=2, N=2048, C=768).

Sharding: the 24 (batch, head) pairs are split across 8 NeuronCores —
4 cores per batch element, 3 heads per core (data + head/tensor parallel).
Each core computes qkv projections for its heads, the full attention for
its heads (the N x N score matrix is private to a core), and a *partial*
output projection over its heads' channels.  The host sums the 4 partial
projections per batch element (the tensor-parallel all-reduce) and adds
the bias.

Device algorithm (activations/weights bf16 — the PE streams its moving
operand at 1 column/cycle for 2-byte dtypes vs 2 cycles for fp32/fp32r —
with fp32 PSUM accumulation everywhere):

  xT [768, 2048] (x transposed on host)
  B:  qk^T  = W_qk^T.T @ xT  -> per-head tile [q^T(64 rows); k^T(64)] x 2048
      (attention scale 1/8 and b_q, b_k folded into W/bias on host)
  B2: v     = xT.T @ W_v^T   -> [2048, 3*65] with a column of ones per head
  C:  S^T[key, q] = k^T.T @ q^T        (per 128-key tile, 512-q chunk)
      P^T = exp(S^T)                   (ScalarE, no max subtraction:
                                        logits are in [-3, 3] by construction)
      ctx_u^T[d|den, q] += [v | 1].T @ P^T   (fused denominator row)
  D:  ctx^T = ctx_u^T[0:64] * (1/den)  (den broadcast across partitions via a
      ones-row matmul at base partition 64, then reciprocal_approx_fast)
  E:  y[n, :] += ctx^T.T @ W_p^T      (partial projection, summed on host)

Scheduling notes (why the emission order looks the way it does): the PE's
HAM clock gate re-throttles 2.4->1.2 GHz after ~3.4us of idle, so the
normalize/projection work for a tile is deferred and emitted mid-way
through the NEXT attention block, where the exp pipeline has buffered
work; a warm-up matmul spin bridges the input-DMA window; the exp table
set is preloaded with a dummy activation; S^T matmul pairs are emitted
back-to-back into opposite PE row groups so they overlap.
"""

import numpy as np
import ml_dtypes

import concourse.bacc as bacc
import concourse.tile as tile
import concourse.mybir as mybir
from concourse.bass_utils import run_bass_kernel_spmd

# Problem shape (hardcoded; harness contract)
B, N, C = 2, 2048, 768
H, HD = 12, 64
NCORES = 8
CORES_PER_B = NCORES // B      # 4
HPC = H // CORES_PER_B         # 3 heads per core
P = 128
NT = N // P                    # 16 key/n tiles
KT = C // P                    # 6 c_in tiles
CH = 512                       # q chunk (max fp32 moving free dim)
QCH = N // CH                  # 4 chunks
VW = 3 * 65                    # v width: 3 heads x (64 + fused ones column)

f32 = mybir.dt.float32
f32r = mybir.dt.float32r
bf16 = mybir.dt.bfloat16
EXP = mybir.ActivationFunctionType.Exp

def _emit(tc, nc, xT, w_qk, w_v, b_qk, w_p, vones, onesrow, y, dbg=None):
    from contextlib import ExitStack

    with ExitStack() as ctx:
        consts = ctx.enter_context(tc.tile_pool(name="consts", bufs=1))
        qk_pool = ctx.enter_context(tc.tile_pool(name="qk", bufs=HPC))
        qk2_pool = ctx.enter_context(tc.tile_pool(name="qk2", bufs=HPC))
        v_pool = ctx.enter_context(tc.tile_pool(name="v", bufs=NT))
        ctx_pool = ctx.enter_context(tc.tile_pool(name="ctxp", bufs=HPC))
        y_pool = ctx.enter_context(tc.tile_pool(name="y", bufs=3))
        ps_a = ctx.enter_context(tc.tile_pool(name="ps_a", bufs=2, space="PSUM"))
        ps_s = ctx.enter_context(tc.tile_pool(name="ps_s", bufs=2, space="PSUM"))
        ps_c = ctx.enter_context(tc.tile_pool(name="ps_c", bufs=2, space="PSUM"))

        # ---- constants
        vones_sb = consts.tile([P, VW], bf16)
        nc.sync.dma_start(vones_sb[:], vones[:])
        wqk_sb = consts.tile([P, KT, 2 * HD * HPC], bf16)
        _wqk = w_qk.rearrange("(t p) m -> p t m", p=P)
        for kt in range(KT):
            nc.sync.dma_start(wqk_sb[:, kt, :], _wqk[:, kt, :])
        wv_sb = consts.tile([P, KT, VW], bf16)
        nc.sync.dma_start(wv_sb[:], w_v.rearrange("(t p) m -> p t m", p=P))
        bqk_sb = consts.tile([P, HPC], f32)
        nc.sync.dma_start(bqk_sb[:], b_qk.rearrange("t p -> p t"))
        wp_sb = consts.tile([HD, HPC, C], bf16)
        nc.sync.dma_start(wp_sb[:], w_p.rearrange("(h p) m -> p h m", p=HD))
        onesrow_sb = consts.tile([HD + 1, HD], bf16)
        nc.sync.dma_start(onesrow_sb[:], onesrow[:])

        # persistent activations
        qk_sb = [qk_pool.tile([P, N], bf16, tag="qk", name=f"qk{_}") for _ in range(HPC)]
        qk2_sb = [qk2_pool.tile([P, N], bf16, tag="qk2", name=f"qk2_{_}") for _ in range(HPC)]
        v_sb = [v_pool.tile([P, VW], bf16, tag="v", name=f"v{_}") for _ in range(NT)]
        ctx_sb = [ctx_pool.tile([HD, N], bf16, tag="ctx", name=f"ctx{_}") for _ in range(HPC)]

        x_pool = ctx.enter_context(tc.tile_pool(name="x", bufs=KT))
        x_sb = [x_pool.tile([P, N], bf16, tag="x", name=f"x{_}") for _ in range(KT)]
        half = N // 2
        for kt in range(KT):
            nc.sync.dma_start(x_sb[kt][:, 0:half], xT[kt * P : (kt + 1) * P, 0:half])
        for kt in range(KT):
            nc.sync.dma_start(
                x_sb[kt][:, half:N], xT[kt * P : (kt + 1) * P, half:N]
            )

        # PE warm-up: the HAM clock gate needs ~3.4us of sustained matmul
        # activity to lift the PE from 1.2 to 2.4 GHz, and re-throttles after
        # a ~3.4us idle window.  Spin dummy matmuls on the (tiny, early) vones
        # tile while the x/weight DMAs land so the real matmuls start warm.
        wps = ps_a.tile([P, CH], f32, tag="ps_a", name="warm_ps")
        for _ in range(75):
            nc.tensor.matmul(
                wps[:, 0:VW], vones_sb[:, 0:P], vones_sb[:], start=True, stop=True
            )
        # preload the exp spline table set (~2.7us) during the DMA window
        actwarm = consts.tile([P, VW], bf16)
        nc.scalar.activation(actwarm[:], vones_sb[:], EXP)

        def emit_qk_group(t, cc):
            # qk^T head tile t, q-chunk cc: [q^T(64); k^T(64)] x CH
            sl = slice(cc * CH, (cc + 1) * CH)
            ps = ps_a.tile([P, CH], f32, tag="ps_a", name="ps_qk")
            for kt in range(KT):
                nc.tensor.matmul(
                    ps[:],
                    wqk_sb[:, kt, t * P : (t + 1) * P],
                    x_sb[kt][:, sl],
                    start=(kt == 0),
                    stop=(kt == KT - 1),
                )
            nc.vector.tensor_scalar_add(qk_sb[t][:, sl], ps[:], bqk_sb[:, t : t + 1])
            # swapped copy per chunk (k^T to partitions 0:64, q^T to 64:128);
            # chunked so the SBUF->SBUF DMA overlaps the next chunk's matmuls
            nc.sync.dma_start(qk2_sb[t][0:HD, sl], qk_sb[t][HD:P, sl])
            nc.sync.dma_start(qk2_sb[t][HD:P, sl], qk_sb[t][0:HD, sl])

        def emit_v(nt):
            # v natural layout [key, 3*65] (+ ones columns)
            ps = ps_a.tile([P, CH], f32, tag="ps_a", name="ps_v")
            for kt in range(KT):
                nc.tensor.matmul(
                    ps[:, 0:VW],
                    x_sb[kt][:, nt * P : (nt + 1) * P],
                    wv_sb[:, kt, :],
                    start=(kt == 0),
                    stop=(kt == KT - 1),
                )
            nc.vector.tensor_add(v_sb[nt][:], ps[:, 0:VW], vones_sb[:])

        recip_pool = ctx.enter_context(tc.tile_pool(name="recip", bufs=2))
        bc_pool = ctx.enter_context(tc.tile_pool(name="bc", bufs=2))
        p_pool = ctx.enter_context(tc.tile_pool(name="p", bufs=8))

        # ---- C/D/E: attention, normalize (deferred one step), projection.
        # S^T pairs go to opposite PE row groups (partitions 0:64 / 64:128)
        # and run concurrently; exp processes both halves in one ACTIVATE.
        # The normalize (D) for tile (c,h) is emitted only after the next
        # C-block so its reciprocal never stalls the PE (a >3.4us PE gap
        # re-throttles the HAM clock gate to 1.2 GHz).
        def emit_C(c, h, first=False, flush=None):
            cps = ps_c.tile([65, CH], f32, tag="ps_c", name="cps")
            for kp in range(NT // 2):
                kt0, kt1 = 2 * kp, 2 * kp + 1
                if first and kp >= 1:
                    # just-in-time v projection (v0/v1 are emitted before the
                    # C loop while the q/k swap DMA is still in flight)
                    emit_v(kt0)
                    emit_v(kt1)
                sps = ps_s.tile([P, 2 * CH], f32, tag="ps_s", name="sps")
                nc.tensor.matmul(
                    sps[:, 0:CH],
                    qk2_sb[h][0:HD, kt0 * P : (kt0 + 1) * P],
                    qk_sb[h][0:HD, c * CH : (c + 1) * CH],
                )
                nc.tensor.matmul(
                    sps[:, CH : 2 * CH],
                    qk_sb[h][HD:P, kt1 * P : (kt1 + 1) * P],
                    qk2_sb[h][HD:P, c * CH : (c + 1) * CH],
                )
                pt = p_pool.tile([P, 2 * CH], bf16, tag="p", name="pt")
                nc.scalar.activation(pt[:], sps[:], EXP)
                nc.tensor.matmul(
                    cps[:],
                    v_sb[kt0][:, h * 65 : (h + 1) * 65],
                    pt[:, 0:CH],
                    start=(kp == 0),
                    stop=False,
                )
                nc.tensor.matmul(
                    cps[:],
                    v_sb[kt1][:, h * 65 : (h + 1) * 65],
                    pt[:, CH : 2 * CH],
                    start=False,
                    stop=(kp == NT // 2 - 1),
                )
                if kp % 2 == 1 and flush is not None:
                    # emit the previous tile's normalize / projection units
                    # mid-block (not at kp 0: ACT has no buffered work at a
                    # block start, so PE detours there starve the exp stream)
                    flush(1)
            return cps

        def emit_D(c, h, cps):
            denr = recip_pool.tile([65, CH], bf16, tag="denr", name="denr")
            nc.vector.tensor_copy(denr[64:65, :], cps[64:65, :])
            bps = ps_a.tile([P, CH], f32, tag="ps_a", name="bps")
            nc.tensor.matmul(
                bps[0:HD, :], onesrow_sb[HD : HD + 1, :], denr[64:65, :],
                start=True, stop=True,
            )
            bcd = bc_pool.tile([HD, CH], f32, tag="bcd", name="bcd")
            nc.vector.tensor_copy(bcd[:], bps[0:HD, :])
            bc = bc_pool.tile([HD, CH], f32, tag="bc", name="bc")
            nc.vector.reciprocal_approx_fast(bc[:], bcd[:])
            nc.vector.tensor_mul(
                ctx_sb[h][:, c * CH : (c + 1) * CH], cps[0:HD, :], bc[:]
            )

        def emit_E_unit(nt):
            # partial projection for one n-tile (summed over this core's heads)
            psA = ps_a.tile([P, CH], f32, tag="ps_a", name="psA")
            psB = ps_a.tile([P, CH], f32, tag="ps_a", name="psB")
            for h in range(HPC):
                nc.tensor.matmul(
                    psA[:],
                    ctx_sb[h][:, nt * P : (nt + 1) * P],
                    wp_sb[:, h, 0:CH],
                    start=(h == 0),
                    stop=(h == HPC - 1),
                )
            for h in range(HPC):
                nc.tensor.matmul(
                    psB[:, 0 : C - CH],
                    ctx_sb[h][:, nt * P : (nt + 1) * P],
                    wp_sb[:, h, CH:C],
                    start=(h == 0),
                    stop=(h == HPC - 1),
                )
            ysb = y_pool.tile([P, C], bf16, tag="y", name="ysb")
            nc.vector.tensor_copy(ysb[:, 0:CH], psA[:])
            nc.vector.tensor_copy(ysb[:, CH:C], psB[:, 0 : C - CH])
            nc.sync.dma_start(y[nt * P : (nt + 1) * P, :], ysb[:])

        pending = [None]  # (c, h, cps) awaiting D
        pending_E = []  # (c, nt) projection units ready to emit

        def flush_pending(budget=1):
            if pending[0] is not None:
                pc, ph, pcps = pending[0]
                emit_D(pc, ph, pcps)
                if ph == HPC - 1:
                    pending_E.extend(
                        pc * (CH // P) + i for i in range(CH // P)
                    )
                pending[0] = None
                budget -= 1
            while budget > 0 and pending_E:
                emit_E_unit(pending_E.pop(0))
                budget -= 1

        # Head-outer schedule: all 4 q-chunks of head h, then head h+1.
        # The next head's qk projection is emitted one chunk-group at a time
        # underneath the current (ACT-bound) attention blocks.
        emit_qk_group(0, 0)
        emit_qk_group(0, 1)
        emit_v(0)
        emit_v(1)
        emit_qk_group(0, 2)
        emit_qk_group(0, 3)
        for h in range(HPC):
            for c in range(QCH):
                cps = emit_C(c, h, first=(h == 0 and c == 0), flush=flush_pending)
                pending[0] = (c, h, cps)
                if h < HPC - 1:
                    emit_qk_group(h + 1, c)
        while pending[0] is not None or pending_E:
            flush_pending()


def build_program(debug=False):
    nc = bacc.Bacc("TRN2", target_bir_lowering=False, debug=False)
    xT = nc.dram_tensor("xT", [C, N], bf16, kind="ExternalInput").ap()
    w_qk = nc.dram_tensor("w_qk", [C, 2 * HD * HPC], bf16, kind="ExternalInput").ap()
    w_v = nc.dram_tensor("w_v", [C, VW], bf16, kind="ExternalInput").ap()
    b_qk = nc.dram_tensor("b_qk", [HPC, P], f32, kind="ExternalInput").ap()
    w_p = nc.dram_tensor("w_p", [HPC * HD, C], bf16, kind="ExternalInput").ap()
    vones = nc.dram_tensor("vones", [P, VW], bf16, kind="ExternalInput").ap()
    onesrow = nc.dram_tensor("onesrow", [HD + 1, HD], bf16, kind="ExternalInput").ap()
    y = nc.dram_tensor("y", [N, C], bf16, kind="ExternalOutput").ap()
    with tile.TileContext(nc) as tc:
        _emit(tc, nc, xT, w_qk, w_v, b_qk, w_p, vones, onesrow, y)
    nc.compile()
    return nc


_CACHE = {}


def _get_program():
    if "nc" not in _CACHE:
        _CACHE["nc"] = build_program()
    return _CACHE["nc"]


def make_in_maps(x, W_qkv, b_qkv, W_proj):
    """Per-core input dicts implementing the (batch, head-group) sharding."""
    x = np.ascontiguousarray(np.asarray(x, np.float32))
    W_qkv = np.asarray(W_qkv, np.float32)
    b_qkv = np.asarray(b_qkv, np.float32)
    W_proj = np.asarray(W_proj, np.float32)
    scale = float(HD) ** -0.5

    Wq = W_qkv[0:C].reshape(H, HD, C)
    Wk = W_qkv[C : 2 * C].reshape(H, HD, C)
    Wv = W_qkv[2 * C : 3 * C].reshape(H, HD, C)
    bq = b_qkv[0:C].reshape(H, HD)
    bk = b_qkv[C : 2 * C].reshape(H, HD)

    vones_mask = np.zeros((P, VW), np.float32)
    for i in range(HPC):
        vones_mask[:, i * 65 + HD] = 1.0
    onesrow_arr = np.zeros((HD + 1, HD), np.float32)
    onesrow_arr[HD, :] = 1.0

    in_maps = []
    for core in range(NCORES):
        b = core // CORES_PER_B
        hg = core % CORES_PER_B
        heads = list(range(hg * HPC, (hg + 1) * HPC))

        xT = np.ascontiguousarray(x[b].T).astype(ml_dtypes.bfloat16)  # [C, N]
        w_qk = np.empty((C, 2 * HD * HPC), np.float32)  # cast to bf16 below
        b_qk_arr = np.empty((HPC, P), np.float32)
        w_v = np.zeros((C, VW), np.float32)
        w_p = np.empty((HPC * HD, C), np.float32)
        for i, h in enumerate(heads):
            w_qk[:, i * P : i * P + HD] = Wq[h].T * scale
            w_qk[:, i * P + HD : (i + 1) * P] = Wk[h].T
            b_qk_arr[i, 0:HD] = bq[h] * scale
            b_qk_arr[i, HD:P] = bk[h]
            w_v[:, i * 65 : i * 65 + HD] = Wv[h].T
            w_p[i * HD : (i + 1) * HD, :] = W_proj[:, h * HD : (h + 1) * HD].T
        in_maps.append(
            {"xT": xT,
             "w_qk": w_qk.astype(ml_dtypes.bfloat16),
             "w_v": w_v.astype(ml_dtypes.bfloat16),
             "b_qk": b_qk_arr,
             "w_p": w_p.astype(ml_dtypes.bfloat16),
             "vones": vones_mask.astype(ml_dtypes.bfloat16),
             "onesrow": onesrow_arr.astype(ml_dtypes.bfloat16)}
        )
    return in_maps


def gather_output(results, b_qkv, W_proj, b_proj):
    """Sum the per-core partial projections (TP all-reduce) + effective bias."""
    out = np.zeros((B, N, C), np.float32)
    for core in range(NCORES):
        out[core // CORES_PER_B] += np.asarray(results[core]["y"], np.float32)
    b_v = np.asarray(b_qkv, np.float32)[2 * C : 3 * C]
    b_eff = np.asarray(b_proj, np.float32) + np.asarray(W_proj, np.float32) @ b_v
    out += b_eff
    return out


def kernel(x=None, xpos=None, W_qkv=None, b_qkv=None, W_proj=None, b_proj=None, **kw):
    del xpos, kw  # rope disabled in this configuration; xpos unused
    nc = _get_program()
    in_maps = make_in_maps(x, W_qkv, b_qkv, W_proj)
    res = run_bass_kernel_spmd(nc, in_maps, core_ids=list(range(NCORES)))
    return gather_output(res.results, b_qkv, W_proj, b_proj)
